# revision 1
# baseline (speedup 1.0000x reference)
"""Trainium2 Bass kernel for nn_ContrastiveSingleProsodyLoss.

loss = mean_a[ log(sum_b exp(2*sim[a,b]) - e^2) - log(nominator[a]) ]
with sim[a,b] = 1/(1+|rep[a]-rep[b]|), rep = concat(emb_i[:,0], emb_j[:,0]),
N = 16384. The device computes the O(N^2) part

    rowsum[a] = sum_b exp(2 / (1 + |rep[a] - rep[b]|))

and the host finishes the O(N) tail in float64.

Distribution (8 NeuronCores, SPMD): 128-row chunks are stride-8 interleaved
across cores, which makes every core's tile structure identical and its
upper-triangle workload exactly equal. sim is symmetric, so each chunk only
computes column blocks at/after its own diagonal block (and odd chunks only
the right half of their diagonal block); the skipped lower-triangle cells
are recovered from per-tile column sums and all parts are combined on the
host.

Per [128, 2048] tile the per-core pipeline is:
  DVE:  s ~= 1/(1+|rep_bcast - rep_a|)  one fused custom DVE op (absdiff,
        +1, exponent-flip seed, one tuned Newton step; 1.7e-3 max rel err
        that cancels to ~4e-5 in the loss)
  ACT:  e = exp(2*s) in bf16, accum_out -> rowsum partials
  PE :  column sums of e (e-slice stationary x ones), PSUM -> DVE add
Measured: ~203 us HW exec, loss rel err 3.8e-05.
"""

import numpy as np

import concourse.bass as bass
import concourse.mybir as mybir
import concourse.tile as tile
from concourse import bacc
from concourse import dve_ops as _dve_ops
from concourse.bass_utils import run_bass_kernel_spmd
from concourse.dve_ops import DveOp
from concourse.dve_spec import C0, C1, C2, Bin, One, Spec, Src0, _has_src1, lower
from concourse.dve_uop import AluOp, DveOpSpec

F32 = mybir.dt.float32

# --- custom fused DVE op: out ~= 1/(1 + |in0 - s0|) ------------------------
# t = |x - r| + 1; seed via fp32 exponent-flip (bitwise NOT); one
# Newton-Raphson step with minimax-tuned constants (max rel err 1.7e-3,
# which cancels to ~4e-5 in the final loss).
RECIP_A = -0.23549784
RECIP_B = 2.00173236

_t = Bin(AluOp.ADD, Bin(AluOp.ABSOLUTE_DIFF, Src0, C0), One)
_nt = Bin(AluOp.BITWISE_NOT, _t, _t)
_y0 = _nt * C1
_recip1p_body = _y0 * (C2 - _t * _y0)


def _ref_recip1p(in0, in1, s0, s1, imm2):
    t = (np.abs(in0 - s0) + np.float32(1.0)).astype(np.float32)
    nt = (~t.view(np.int32)).view(np.float32)
    y0 = (nt * np.float32(s1)).astype(np.float32)
    return (y0 * (np.float32(imm2) - t * y0)).astype(np.float32)


def _register_recip1p() -> DveOp:
    name = "RECIP1P_ABSDIFF_ANT"
    for op in _dve_ops.OPS:
        if op.name == name:
            return op
    row = max(_dve_ops._SUB_OPCODE_FOR_NAME.values()) + 1
    assert row < 0x20
    _dve_ops._SUB_OPCODE_FOR_NAME[name] = row
    spec = Spec(body=_recip1p_body, reference=_ref_recip1p)
    shas = {}
    for ver in ("v3", "v4"):
        uops = lower(spec, ver=ver)
        shas[ver] = DveOpSpec(
            name=name, opcode=row, uops=uops, rd1_en=_has_src1(spec)
        ).sha(ver)
    op = DveOp(name, spec, subdim=False, uops_sha=shas)
    _dve_ops.OPS.append(op)
    _dve_ops.CUSTOM_DVE_SPECS[name] = spec
    return op


RECIP1P = _register_recip1p()

B = 8192
N = 2 * B
NCORES = 8
RPC = N // NCORES  # rows per core
P = 128
FD = 2048  # free-dim chunk per DVE/ACT instruction

TEMPERATURE = 0.5
EPS = 0.01

TRACE = False
TRACE_DIR = None
LAST_RESULTS = None


def build_program(n=N, rpc=RPC, fd=FD):
    nc = bacc.Bacc(trn_type="TRN2")
    rep_h = nc.declare_dram_parameter("rep", [n], F32, isOutput=False)
    repa_h = nc.declare_dram_parameter("repa", [rpc], F32, isOutput=False)
    out_h = nc.declare_dram_parameter("rowsum", [rpc], F32, isOutput=True)

    ncc = n // fd
    nrc = rpc // P

    with tile.TileContext(nc) as tc:
        with (
            tc.tile_pool(name="singles", bufs=1) as singles,
            tc.tile_pool(name="work", bufs=2) as work,
            tc.tile_pool(name="spool", bufs=3) as spool,
        ):
            # this core's row values, laid out [P, nrc]: column j holds rows
            # j*128 .. j*128+127
            repa_t = singles.tile([P, nrc], F32, tag="repa")
            nc.sync.dma_start(
                out=repa_t[:], in_=repa_h[:].rearrange("(n p) -> p n", p=P)
            )

            # rep broadcast across partitions, one tile per column chunk
            bcs = []
            for cc in range(ncc):
                t = singles.tile([P, fd], F32, tag=f"bc{cc}")
                src = rep_h[cc * fd : (cc + 1) * fd]
                bsrc = bass.AP(
                    tensor=src.tensor,
                    offset=src.offset,
                    ap=[[0, P]] + [list(x) for x in src.ap],
                )
                nc.sync.dma_start(out=t[:], in_=bsrc)
                bcs.append(t)

            acc = singles.tile([P, nrc * ncc], F32, tag="acc")
            rsall = singles.tile([P, nrc], F32, tag="rsall")

            for cc in range(ncc):
                for rc in range(nrc):
                    s = spool.tile([P, fd], F32, tag="s")
                    nc.vector._custom_dve(
                        RECIP1P,
                        out=s[:],
                        in0=bcs[cc][:],
                        s0=repa_t[:, rc : rc + 1],
                        s1=RECIP_A,
                        imm2=RECIP_B,
                    )
                    e = work.tile([P, fd], F32, tag="e")
                    nc.scalar.activation(
                        out=e[:],
                        in_=s[:],
                        func=mybir.ActivationFunctionType.Exp,
                        bias=0.0,
                        scale=2.0,
                        accum_out=acc[:, rc * ncc + cc : rc * ncc + cc + 1],
                    )

            for rc in range(nrc):
                nc.vector.tensor_reduce(
                    out=rsall[:, rc : rc + 1],
                    in_=acc[:, rc * ncc : (rc + 1) * ncc],
                    axis=mybir.AxisListType.X,
                    op=mybir.AluOpType.add,
                )
            nc.sync.dma_start(
                out=out_h[:].rearrange("(n p) -> p n", p=P), in_=rsall[:]
            )
    nc.compile()
    return nc


def core_chunks(c, nchunks=N // P):
    """Global 128-row chunk ids owned by core c. Stride-NCORES interleaving
    makes the per-chunk column-block indices (and hence the whole tile
    structure and upper-triangle workload) identical for every core, so one
    SPMD program serves all cores."""
    return [c + NCORES * t for t in range(nchunks // NCORES)]


def build_program_v3(n=N, rpc=RPC, fd=FD, core=0):
    """Symmetric (upper-triangle) version: each core computes tiles with
    column block >= its chunk's block; lower-triangle contributions are
    recovered from per-tile column sums (PE matmul with a ones vector over
    the bf16 exp tile) accumulated in PSUM and all combined on the host.

    The chunk->tile structure is identical for every core (the tile loop
    below only depends on block16 indices, which are the same for all cores
    by the pairing symmetry), so one SPMD program serves all cores.
    """
    BF16 = mybir.dt.bfloat16
    nc = bacc.Bacc(trn_type="TRN2")
    rep_h = nc.declare_dram_parameter("rep", [n], F32, isOutput=False)
    repa_h = nc.declare_dram_parameter("repa", [rpc], F32, isOutput=False)
    onesb_h = nc.declare_dram_parameter("onesb", [P], BF16, isOutput=False)
    out_h = nc.declare_dram_parameter("rowsum", [rpc], F32, isOutput=True)
    colsum_h = nc.declare_dram_parameter("colsum", [n], F32, isOutput=True)

    ncc = n // fd
    nrc = rpc // P
    G = core_chunks(core, n // P)
    blocks = [g * P // fd for g in G]

    with tile.TileContext(nc) as tc:
        with (
            tc.tile_pool(name="singles", bufs=1) as singles,
            tc.tile_pool(name="work", bufs=3) as work,
            tc.tile_pool(name="spool", bufs=5) as spool,
            tc.tile_pool(name="psum", bufs=2, space="PSUM") as psum,
        ):
            repa_t = singles.tile([P, nrc], F32, tag="repa")
            nc.sync.dma_start(
                out=repa_t[:], in_=repa_h[:].rearrange("(n p) -> p n", p=P)
            )
            onesb_t = singles.tile([P, 1], BF16, tag="onesb")
            nc.sync.dma_start(out=onesb_t[:], in_=onesb_h[:, None])

            bcs = []
            for cc in range(ncc):
                t = singles.tile([P, fd], F32, tag=f"bc{cc}")
                src = rep_h[cc * fd : (cc + 1) * fd]
                bsrc = bass.AP(
                    tensor=src.tensor,
                    offset=src.offset,
                    ap=[[0, P]] + [list(x) for x in src.ap],
                )
                nc.sync.dma_start(out=t[:], in_=bsrc)
                bcs.append(t)

            acc = singles.tile([P, nrc * ncc], F32, tag="acc")
            rsall = singles.tile([P, nrc], F32, tag="rsall")

            half = fd // 2
            for cb in range(ncc):
                todo = [k for k in range(nrc) if blocks[k] <= cb]
                # per-cb SBUF accumulator for the column sums; zeroed on Pool
                # (idle engine), all contributors then add into it
                csb = work.tile([P, fd // P], F32, tag="csb")
                nc.gpsimd.memset(csb[:], 0.0)
                for k in todo:
                    is_d = blocks[k] == cb
                    # odd-t chunks sit in the right half of their block, so
                    # their diagonal tile only needs columns [half, fd); the
                    # skipped left-half cells are recovered by symmetry from
                    # the even-t D-tiles' right-half column sums below
                    off = half if (is_d and k % 2 == 1) else 0
                    w = fd - off
                    s = spool.tile([P, fd], F32, tag="s")
                    nc.vector._custom_dve(
                        RECIP1P,
                        out=s[:, :w],
                        in0=bcs[cb][:, off:],
                        s0=repa_t[:, k : k + 1],
                        s1=RECIP_A,
                        imm2=RECIP_B,
                    )
                    e = work.tile([P, fd], BF16, tag="e")
                    nc.scalar.activation(
                        out=e[:, :w],
                        in_=s[:, :w],
                        func=mybir.ActivationFunctionType.Exp,
                        bias=0.0,
                        scale=2.0,
                        accum_out=acc[:, k * ncc + cb : k * ncc + cb + 1],
                    )
                    if not is_d:
                        jlo, jhi = 0, fd // P  # U-tile: all column slices
                    elif k % 2 == 0:
                        jlo, jhi = half // P, fd // P  # even D: right half
                    else:
                        jlo = jhi = 0  # odd D: no colsum
                    if jlo < jhi:
                        # colsum across partitions: for 128-column slice j,
                        # out[m, 0] = sum_p E[p, j*128+m] (E slice is the
                        # stationary operand); fresh PSUM per tile, then a
                        # tiny DVE add into the SBUF accumulator
                        cs = psum.tile([P, fd // P], F32, tag="colsum")
                        for j in range(jlo, jhi):
                            nc.tensor.matmul(
                                cs[:, j : j + 1],
                                e[:, j * P : (j + 1) * P],
                                onesb_t[:],
                                start=True,
                                stop=True,
                            )
                        nc.vector.tensor_tensor(
                            csb[:, jlo:jhi],
                            csb[:, jlo:jhi],
                            cs[:, jlo:jhi],
                            mybir.AluOpType.add,
                        )
                nc.sync.dma_start(
                    out=colsum_h[cb * fd : (cb + 1) * fd].rearrange(
                        "(j p) -> p j", p=P
                    ),
                    in_=csb[:],
                )

            for k in range(nrc):
                lo = k * ncc + blocks[k]
                hi = (k + 1) * ncc
                nc.vector.tensor_reduce(
                    out=rsall[:, k : k + 1],
                    in_=acc[:, lo:hi],
                    axis=mybir.AxisListType.X,
                    op=mybir.AluOpType.add,
                )
            nc.sync.dma_start(
                out=out_h[:].rearrange("(n p) -> p n", p=P), in_=rsall[:]
            )
    nc.compile()
    return nc


_CACHE = {}
USE_V3 = True


def _get_nc():
    key = "nc3" if USE_V3 else "nc"
    if key not in _CACHE:
        _CACHE[key] = build_program_v3() if USE_V3 else build_program()
    return _CACHE[key]


def _run_v3(rep):
    import ml_dtypes

    nc = _get_nc()
    onesb = np.ones(P, dtype=ml_dtypes.bfloat16)
    in_maps = []
    for c in range(NCORES):
        repa = np.concatenate(
            [rep[g * P : (g + 1) * P] for g in core_chunks(c)]
        )
        in_maps.append({"rep": rep, "repa": repa, "onesb": onesb})
    res = run_bass_kernel_spmd(
        nc, in_maps, list(range(NCORES)), trace=TRACE, tmpdir=TRACE_DIR
    )
    rowsum = np.zeros(N, np.float64)
    for c in range(NCORES):
        rs = res.results[c]["rowsum"].astype(np.float64)
        for k, g in enumerate(core_chunks(c)):
            rowsum[g * P : (g + 1) * P] = rs[k * P : (k + 1) * P]
    for c in range(NCORES):
        rowsum += res.results[c]["colsum"].astype(np.float64)
    return rowsum, res


def _finalize(rowsum, emb_i, emb_j, prosody_i, prosody_j):
    """O(N) tail in float64 on host."""
    den = rowsum.astype(np.float64) - np.exp(2.0)
    ei = np.asarray(emb_i, np.float64)[:, 0]
    ej = np.asarray(emb_j, np.float64)[:, 0]
    p = 1.0 / (1.0 + np.abs(ej - ei))
    positives = np.concatenate([p, p])
    pd = np.abs(
        np.asarray(prosody_i, np.float64) - np.asarray(prosody_j, np.float64)
    )
    sm = np.exp(pd - pd.max())
    sm /= sm.sum()
    prosody = np.concatenate([sm, sm]) + EPS
    nominator = positives / prosody
    loss = np.mean(np.log(den) - np.log(nominator))
    return np.asarray(loss, dtype=np.float32)


def kernel(emb_i, emb_j, prosody_i, prosody_j):
    global LAST_RESULTS
    emb_i = np.asarray(emb_i)
    emb_j = np.asarray(emb_j)
    rep = np.concatenate([emb_i[:, 0], emb_j[:, 0]]).astype(np.float32)
    if USE_V3:
        rowsum, res = _run_v3(rep)
    else:
        nc = _get_nc()
        in_maps = [
            {
                "rep": rep,
                "repa": np.ascontiguousarray(rep[c * RPC : (c + 1) * RPC]),
            }
            for c in range(NCORES)
        ]
        res = run_bass_kernel_spmd(
            nc, in_maps, list(range(NCORES)), trace=TRACE
        )
        rowsum = np.concatenate(
            [res.results[c]["rowsum"] for c in range(NCORES)]
        )
    LAST_RESULTS = res
    return _finalize(rowsum, emb_i, emb_j, prosody_i, prosody_j)



# revision 4
# speedup vs baseline: 7.6572x; 7.6572x over previous
"""Trainium2 Bass kernel for nn_ContrastiveSingleProsodyLoss.

loss = mean_a[ log(sum_b exp(2*sim[a,b]) - e^2) - log(nominator[a]) ]
with sim[a,b] = 1/(1+|rep[a]-rep[b]|), rep = concat(emb_i[:,0], emb_j[:,0]),
N = 16384.

The O(N^2) pairwise sum is a 1-D kernel sum over the scalar rep values:
rowsum[a] = sum_b g(rep[a]-rep[b]), g(d) = exp(2/(1+|d|)). The host bins
the N rep values onto NBINS uniformly spaced centers with linear binning
(mass split between the two neighboring centers; O(width^2) accurate,
loss rel err ~4e-7 at NBINS=256), so the device only evaluates

    rowsum[a] ~= sum_j H[j] * g(c_j - rep[a])        [NBINS x N elements]

a 64x element reduction vs the explicit half-triangle. Each of the 8
cores takes a contiguous 2048-slice of a.

Per core, per 512-wide a-chunk (bins on partitions, a on free dim):
  DVE:  s = 1/(1+|x - c_p|) fused custom op (absdiff, +1, exponent-flip
        seed, one Newton step)
  ACT:  e = exp(2*s) in bf16
  PE :  psum[1, 512] += H_blk^T @ e  (hist-weighted partition reduction)
  Pool: psum -> sbuf copy
and the host finishes the O(N) tail in float64.
"""

import numpy as np

import concourse.bass as bass
import concourse.mybir as mybir
import concourse.tile as tile
from concourse import bacc
from concourse import dve_ops as _dve_ops
from concourse.bass_utils import run_bass_kernel_spmd
from concourse.dve_ops import DveOp
from concourse.dve_spec import C0, C1, C2, Bin, One, Spec, Src0, _has_src1, lower
from concourse.dve_uop import AluOp, DveOpSpec

F32 = mybir.dt.float32
BF16 = mybir.dt.bfloat16

# --- custom fused DVE op: out ~= 1/(1 + |in0 - s0|) ------------------------
# t = |x - c| + 1; seed via fp32 exponent-flip (bitwise NOT); one
# Newton-Raphson step with minimax-tuned constants (max rel err 1.7e-3).
RECIP_A = -0.23549784
RECIP_B = 2.00173236

_t = Bin(AluOp.ADD, Bin(AluOp.ABSOLUTE_DIFF, Src0, C0), One)
_nt = Bin(AluOp.BITWISE_NOT, _t, _t)
_y0 = _nt * C1
_recip1p_body = _y0 * (C2 - _t * _y0)


def _ref_recip1p(in0, in1, s0, s1, imm2):
    t = (np.abs(in0 - s0) + np.float32(1.0)).astype(np.float32)
    nt = (~t.view(np.int32)).view(np.float32)
    y0 = (nt * np.float32(s1)).astype(np.float32)
    return (y0 * (np.float32(imm2) - t * y0)).astype(np.float32)


def _register_recip1p() -> DveOp:
    name = "RECIP1P_ABSDIFF_ANT"
    for op in _dve_ops.OPS:
        if op.name == name:
            return op
    row = max(_dve_ops._SUB_OPCODE_FOR_NAME.values()) + 1
    assert row < 0x20
    _dve_ops._SUB_OPCODE_FOR_NAME[name] = row
    spec = Spec(body=_recip1p_body, reference=_ref_recip1p)
    shas = {}
    for ver in ("v3", "v4"):
        uops = lower(spec, ver=ver)
        shas[ver] = DveOpSpec(
            name=name, opcode=row, uops=uops, rd1_en=_has_src1(spec)
        ).sha(ver)
    op = DveOp(name, spec, subdim=False, uops_sha=shas)
    _dve_ops.OPS.append(op)
    _dve_ops.CUSTOM_DVE_SPECS[name] = spec
    return op


RECIP1P = _register_recip1p()

B = 8192
N = 2 * B
NCORES = 8
RPC = N // NCORES  # a-values per core
P = 128
NBINS = 256
NBLK = NBINS // P  # bin blocks on partitions
FD = 512  # a-chunk width (= one PSUM bank of fp32)
NCH = RPC // FD

TEMPERATURE = 0.5
EPS = 0.01

TRACE = False
TRACE_DIR = None
LAST_RESULTS = None


def build_program():
    nc = bacc.Bacc(trn_type="TRN2")
    repx_h = nc.declare_dram_parameter("repx", [RPC], F32, isOutput=False)
    cen_h = nc.declare_dram_parameter("centers", [NBINS], F32, isOutput=False)
    h_h = nc.declare_dram_parameter("hist", [NBINS], BF16, isOutput=False)
    out_h = nc.declare_dram_parameter("rowsum", [RPC], F32, isOutput=True)

    with tile.TileContext(nc) as tc:
        with (
            tc.tile_pool(name="singles", bufs=1) as singles,
            tc.tile_pool(name="spool", bufs=3) as spool,
            tc.tile_pool(name="epool", bufs=3) as epool,
            tc.tile_pool(name="psum", bufs=2, space="PSUM") as psum,
        ):
            # bin centers / weights, column b holds bins b*128 .. b*128+127
            cen_t = singles.tile([P, NBLK], F32, tag="cen")
            nc.sync.dma_start(
                out=cen_t[:], in_=cen_h[:].rearrange("(n p) -> p n", p=P)
            )
            h_t = singles.tile([P, NBLK], BF16, tag="hist")
            nc.sync.dma_start(
                out=h_t[:], in_=h_h[:].rearrange("(n p) -> p n", p=P)
            )

            # this core's a-values broadcast across partitions, per chunk
            xs = []
            for ch in range(NCH):
                t = singles.tile([P, FD], F32, tag=f"x{ch}")
                src = repx_h[ch * FD : (ch + 1) * FD]
                bsrc = bass.AP(
                    tensor=src.tensor,
                    offset=src.offset,
                    ap=[[0, P]] + [list(x) for x in src.ap],
                )
                nc.sync.dma_start(out=t[:], in_=bsrc)
                xs.append(t)

            rs_t = singles.tile([1, RPC], F32, tag="rs")

            for ch in range(NCH):
                ps = psum.tile([1, FD], F32, tag="ps")
                for blk in range(NBLK):
                    s = spool.tile([P, FD], F32, tag="s")
                    nc.vector._custom_dve(
                        RECIP1P,
                        out=s[:],
                        in0=xs[ch][:],
                        s0=cen_t[:, blk : blk + 1],
                        s1=RECIP_A,
                        imm2=RECIP_B,
                    )
                    e = epool.tile([P, FD], BF16, tag="e")
                    nc.scalar.activation(
                        out=e[:],
                        in_=s[:],
                        func=mybir.ActivationFunctionType.Exp,
                        bias=0.0,
                        scale=2.0,
                    )
                    nc.tensor.matmul(
                        ps[:],
                        h_t[:, blk : blk + 1],
                        e[:],
                        start=(blk == 0),
                        stop=(blk == NBLK - 1),
                    )
                nc.scalar.copy(
                    out=rs_t[:, ch * FD : (ch + 1) * FD], in_=ps[:]
                )

            nc.sync.dma_start(out=out_h[None, :], in_=rs_t[:])
    nc.compile()
    return nc


_CACHE = {}


def _get_nc():
    if "nc" not in _CACHE:
        _CACHE["nc"] = build_program()
    return _CACHE["nc"]


def _bin_rep(rep):
    """Linear binning of rep values onto NBINS uniform centers (float64)."""
    r = rep.astype(np.float64)
    lo, hi = r.min(), r.max()
    width = (hi - lo) / (NBINS - 1)
    pos = (r - lo) / width
    j0 = np.floor(pos).astype(np.int64)
    np.clip(j0, 0, NBINS - 2, out=j0)
    frac = pos - j0
    hist = np.zeros(NBINS)
    np.add.at(hist, j0, 1.0 - frac)
    np.add.at(hist, j0 + 1, frac)
    centers = lo + width * np.arange(NBINS)
    return centers.astype(np.float32), hist


def _run_binned(rep):
    import ml_dtypes

    nc = _get_nc()
    centers, hist = _bin_rep(rep)
    histb = hist.astype(ml_dtypes.bfloat16)
    in_maps = [
        {
            "repx": np.ascontiguousarray(rep[c * RPC : (c + 1) * RPC]),
            "centers": centers,
            "hist": histb,
        }
        for c in range(NCORES)
    ]
    res = run_bass_kernel_spmd(
        nc, in_maps, list(range(NCORES)), trace=TRACE, tmpdir=TRACE_DIR
    )
    rowsum = np.concatenate(
        [res.results[c]["rowsum"] for c in range(NCORES)]
    ).astype(np.float64)
    return rowsum, res


def _finalize(rowsum, emb_i, emb_j, prosody_i, prosody_j):
    """O(N) tail in float64 on host."""
    den = rowsum.astype(np.float64) - np.exp(2.0)
    ei = np.asarray(emb_i, np.float64)[:, 0]
    ej = np.asarray(emb_j, np.float64)[:, 0]
    p = 1.0 / (1.0 + np.abs(ej - ei))
    positives = np.concatenate([p, p])
    pd = np.abs(
        np.asarray(prosody_i, np.float64) - np.asarray(prosody_j, np.float64)
    )
    sm = np.exp(pd - pd.max())
    sm /= sm.sum()
    prosody = np.concatenate([sm, sm]) + EPS
    nominator = positives / prosody
    loss = np.mean(np.log(den) - np.log(nominator))
    return np.asarray(loss, dtype=np.float32)


def kernel(emb_i, emb_j, prosody_i, prosody_j):
    global LAST_RESULTS
    emb_i = np.asarray(emb_i)
    emb_j = np.asarray(emb_j)
    rep = np.concatenate([emb_i[:, 0], emb_j[:, 0]]).astype(np.float32)
    rowsum, res = _run_binned(rep)
    LAST_RESULTS = res
    return _finalize(rowsum, emb_i, emb_j, prosody_i, prosody_j)
